# revision 1
# baseline (speedup 1.0000x reference)
"""Gaussian-kernel (Nadaraya-Watson) regression on 8 TRN2 NeuronCores.

Reference computes, for each query q (B=256) and output dim d (3):
    out[q,d] = sum_n Y[n]*K[n,q,d] / sum_n K[n,q,d]
    K[n,q,d] = exp(-0.5*((proj[n,d]-xw[q,d])/H)^2),  H=0.5
with proj = train_X @ W.T  [N,3],  xw = x @ W.T  [B,3],  N=200000.

Device strategy (per core, N-shard of 25000 padded to 25088 = 196*128):
  exponent = -2*(p-q)^2 = 4*p*q - 2*p^2 - 2*q^2  -> a single K=7 matmul:
    lhsT (stationary, per 128-sample chunk) rows: [p_d | 1 | -2*p_d^2]
    rhs  (moving, constant [7, 768], f = q*3+d) rows: [4*xw*delta | -2*xw^2 | delta]
  then one ScalarE Exp pass [128,768] PSUM->SBUF, then a K=128 matmul with
  lhsT=[1|Y] accumulating [2,768] (down,up) in PSUM across all chunks.
Host: shards inputs, sums the 8 partial [2,768] results, returns up/down.
"""

import os
from contextlib import ExitStack

import numpy as np

import concourse.bass as bass
import concourse.tile as tile
from concourse import mybir
from concourse.bass_utils import run_bass_kernel_spmd

N_CORES = 8
B = 256
D = 3
H = 0.5
F = B * D  # 768, free layout f = q*3 + d
N_TOTAL = 200000
N_SHARD = N_TOTAL // N_CORES  # 25000
CHUNK = 128
N_CHUNKS = (N_SHARD + CHUNK - 1) // CHUNK  # 196
N_PAD = N_CHUNKS * CHUNK  # 25088
PAD_P = 20.0  # pad proj value -> exp(-2*(20-q)^2) underflows to exactly 0.0

_nc_cache = {}

# test.py introspection: last BassKernelResults from run_bass_kernel_spmd
LAST_RESULTS = None


def _build_nc():
    f32 = mybir.dt.float32
    nc = bass.Bass(trn_type="TRN2")
    # AR = [R1 | A] merged so the loop's first LDWEIGHTS waits on ONE dma sem
    # (the S3_LW struct only carries a single sync-wait command).
    AR_d = nc.dram_tensor("AR", [7, F + N_PAD], f32, kind="ExternalInput")
    Y2_d = nc.dram_tensor("Y2", [CHUNK, 2 * N_CHUNKS], f32, kind="ExternalInput")
    out_d = nc.dram_tensor("out", [2, F], f32, kind="ExternalOutput")

    f32r = mybir.dt.float32r
    with ExitStack() as ctx:
        tc = ctx.enter_context(tile.TileContext(nc))
        const = ctx.enter_context(tc.tile_pool(name="const", bufs=1))
        kpool = ctx.enter_context(tc.tile_pool(name="kpool", bufs=3))
        dpool = ctx.enter_context(tc.tile_pool(name="dpool", bufs=2, space="PSUM"))
        apool = ctx.enter_context(tc.tile_pool(name="apool", bufs=1, space="PSUM"))

        AR_t = const.tile([7, F + N_PAD], f32r)
        nc.gpsimd.dma_start(out=AR_t[:], in_=AR_d[:])
        Y2_t = const.tile([CHUNK, 2 * N_CHUNKS], f32r)
        nc.gpsimd.dma_start(out=Y2_t[:], in_=Y2_d[:])

        acc0 = apool.tile([2, 512], f32)
        acc1 = apool.tile([2, F - 512], f32)

        # All matmul operands are bitcast to float32r: the PE streams f32r at
        # 1 col/cycle vs plain f32's 4 (two half-rate passes). ~TF32 mantissa;
        # the K error cancels in up/down since both share the same k_t.
        GRP = 2  # chunks per ACT instruction (amortize PSUM access latency)
        N_GRP = N_CHUNKS // GRP  # 98
        FG = F * GRP  # 1536 cols per group tile (3 PSUM banks)

        # Matmul PSUM writes must not cross a 2KB bank boundary (512 f32).
        # Pieces are cut on the 512-col bank grid, the 768-col chunk grid,
        # and the chunk-local 512 grid (acc0/acc1 split). Each piece is
        # >=256 cols so f32r streams at full rate.
        # (start, width, local_offset_within_chunk)
        PIECES = []
        cuts = sorted(
            {m * 512 for m in range(FG // 512 + 1)}
            | {j * F for j in range(GRP + 1)}
            | {j * F + 512 for j in range(GRP)}
        )
        for s, e in zip(cuts[:-1], cuts[1:]):
            PIECES.append((s, e - s))

        def emit_mm1(g, diff):
            for s, w in PIECES:
                j = s // F
                loc = s - j * F
                lhsT1 = AR_t[
                    :, F + (g * GRP + j) * CHUNK : F + (g * GRP + j + 1) * CHUNK
                ]
                nc.tensor.matmul(
                    diff[:, s : s + w],
                    lhsT=lhsT1,
                    rhs=AR_t[:, loc : loc + w],
                    start=True,
                    stop=True,
                )

        def emit_mm2(g, k_t):
            for s, w in PIECES:
                j = s // F
                c = g * GRP + j
                loc = s - j * F
                lhsT2 = Y2_t[:, 2 * c : 2 * c + 2]
                acc, aoff = (acc0, loc) if loc < 512 else (acc1, loc - 512)
                nc.tensor.matmul(
                    acc[:, aoff : aoff + w],
                    lhsT=lhsT2,
                    rhs=k_t[:, s : s + w],
                    start=(c == 0),
                    stop=(c == N_CHUNKS - 1),
                )

        # Software pipeline: emit group g's reduction (mm2) AFTER group g+1's
        # mm1 so the in-order PE queue never blocks on ACT(g) before starting
        # mm1(g+1) — PE and ACT overlap across groups.
        pend = None  # (group, k_t) awaiting reduction
        for g in range(N_GRP):
            diff = dpool.tile([CHUNK, FG], f32)
            emit_mm1(g, diff)
            if pend is not None:
                pg, pk = pend
                emit_mm2(pg, pk)
            k_t = kpool.tile([CHUNK, FG], f32r)
            nc.scalar.activation(k_t[:], diff[:], mybir.ActivationFunctionType.Exp)
            pend = (g, k_t)
        pg, pk = pend
        emit_mm2(pg, pk)

        o_t = const.tile([2, F], f32)
        nc.vector.tensor_copy(o_t[:, 0:512], acc0[:])
        nc.vector.tensor_copy(o_t[:, 512:F], acc1[:])
        nc.gpsimd.dma_start(out=out_d[:], in_=o_t[:])

    _strip_self_waits(nc)
    _split_multi_waits(nc)
    return nc


def _split_multi_waits(nc):
    """Walrus encodes at most one sync-wait per instruction on this target.

    Move all but the last wait of any multi-wait instruction onto preceding
    same-engine NoOps (in-order queues make sequential waiting equivalent to
    the ANDed wait set).
    """
    import bass_rust

    for bb_holder in nc.main_func.blocks:
        insts = list(bb_holder.instructions)
        out = []
        changed = False
        for i in insts:
            si = getattr(i, "sync_info", None)
            if (
                si is not None
                and len(si.on_wait) > 1
                and type(i).__name__ != "InstEventSemaphore"
            ):
                for w in si.on_wait[:-1]:
                    nop = mybir.InstNoOp(
                        name=nc.get_next_instruction_name(),
                        sync_info=bass_rust.SyncInfo(on_wait=[w], on_update=[]),
                        bass_nofuse=True,
                        engine=i.engine,
                    )
                    out.append(nop)
                i.sync_info = bass_rust.SyncInfo(
                    on_wait=[si.on_wait[-1]], on_update=list(si.on_update)
                )
                changed = True
            out.append(i)
        if changed:
            _replace_bb_instructions(bb_holder, out)


def _replace_bb_instructions(bb_holder, new_insts):
    bb = getattr(bb_holder, "bb", bb_holder)
    try:
        bb.instructions = new_insts
    except Exception:
        while len(bb.instructions):
            bb.instructions.pop()
        for x in new_insts:
            bb.add_instruction(x)


def _strip_self_waits(nc):
    """Drop semaphore waits that an in-order engine holds against itself.

    Tile emits WAW waits (e.g. ACT chunk c vs ACT chunk c-bufs reusing a pool
    slot) on the engine's own semaphore. The ACT queue executes in order, so
    these are always satisfied — but they push the per-instruction sync-wait
    count past what the S3D3_AC struct encodes, failing walrus codegen.
    Only waits on semaphores updated exclusively by same-engine instructions
    are removed, and only for the Activation engine (PE reorders LDWEIGHTS).
    """
    import bass_rust

    insts = [i for bb in nc.main_func.blocks for i in bb.instructions]
    updaters = {}
    for i in insts:
        si = getattr(i, "sync_info", None)
        if si is None:
            continue
        for u in si.on_update:
            updaters.setdefault(u.id, set()).add(i.engine)
    for i in insts:
        if i.engine != mybir.EngineType.Activation:
            continue
        si = getattr(i, "sync_info", None)
        if si is None or len(si.on_wait) <= 1:
            continue
        keep = [
            w
            for w in si.on_wait
            if updaters.get(w.id, {None}) != {i.engine}
        ]
        if len(keep) != len(si.on_wait):
            i.sync_info = bass_rust.SyncInfo(
                on_wait=keep, on_update=list(si.on_update)
            )


def _get_nc():
    if "nc" not in _nc_cache:
        _nc_cache["nc"] = _build_nc()
    return _nc_cache["nc"]


def kernel(x, train_X, Y, W):
    global LAST_RESULTS
    x = np.ascontiguousarray(np.asarray(x, dtype=np.float32))
    train_X = np.ascontiguousarray(np.asarray(train_X, dtype=np.float32))
    Y = np.ascontiguousarray(np.asarray(Y, dtype=np.float32))
    W = np.ascontiguousarray(np.asarray(W, dtype=np.float32))

    xw = x @ W.T  # [B,3]
    proj = train_X @ W.T  # [N,3]

    # rhs constant [7, F]: rows 0-2: 4*xw[q,d]*delta(d',d); row 3: -2*xw^2;
    # rows 4-6: delta(d',d)
    R1 = np.zeros((7, B, D), dtype=np.float32)
    for d in range(D):
        R1[d, :, d] = 4.0 * xw[:, d]
        R1[4 + d, :, d] = 1.0
    R1[3] = -2.0 * xw * xw
    R1 = np.ascontiguousarray(R1.reshape(7, F))

    in_maps = []
    for s in range(N_CORES):
        pj = np.full((N_PAD, D), PAD_P, dtype=np.float32)
        pj[:N_SHARD] = proj[s * N_SHARD : (s + 1) * N_SHARD]
        A = np.empty((7, F + N_PAD), dtype=np.float32)
        A[:, 0:F] = R1
        A[0:3, F:] = pj.T
        A[3, F:] = 1.0
        A[4:7, F:] = -2.0 * (pj.T * pj.T)

        y2 = np.zeros((N_PAD, 2), dtype=np.float32)
        y2[:N_SHARD, 0] = 1.0
        y2[:N_SHARD, 1] = Y[s * N_SHARD : (s + 1) * N_SHARD]
        # SBUF image [128, 2*N_CHUNKS]: Y2[p, 2c+t] = y2[c*128+p, t]
        Y2 = np.ascontiguousarray(
            y2.reshape(N_CHUNKS, CHUNK, 2).transpose(1, 0, 2).reshape(CHUNK, -1)
        )
        in_maps.append({"AR": A, "Y2": Y2})

    nc = _get_nc()
    res = run_bass_kernel_spmd(
        nc,
        in_maps,
        core_ids=list(range(N_CORES)),
        trace=bool(int(os.environ.get("KNN_TRACE", "0"))),
    )
    LAST_RESULTS = res

    tot = np.zeros((2, F), dtype=np.float64)
    for r in res.results:
        tot += r["out"].astype(np.float64)
    down = tot[0].reshape(B, D)
    up = tot[1].reshape(B, D)
    return (up / down).astype(np.float32)



# revision 3
# speedup vs baseline: 4.4805x; 4.4805x over previous
"""Gaussian-kernel (Nadaraya-Watson) regression on 8 TRN2 NeuronCores.

Reference: out[q,d] = sum_n Y[n]*K / sum_n K, K = exp(-2*(proj[n,d]-xw[q,d])^2),
proj = train_X @ W.T [N,3], xw = x @ W.T [B,3], N=200000, B=256, H=0.5.

Algorithm (Fourier / fast-Gauss): periodize the 1-D kernel with period P and
truncate its cosine series at M terms:
    exp(-2*D^2) ~= sum_m a_m cos(w_m D),  w_m = 2*pi*m/P
    cos(w_m (p-c)) = cos(w_m p)cos(w_m c) + sin(w_m p)sin(w_m c)
so each core only computes trig MOMENTS of its N-shard:
    mom[w, (m,phi,d)] = sum_n {1,y_n} * {cos,sin}(w_m * proj[n,d])
(2*2*M*D = 204 numbers), and the host combines the 8 partial moments and
evaluates the tiny [B,3] query-side sum in f64. With P=12, M=17 the rel
error is ~1e-3 (fp16-quantized pipeline, numpy-simulated).

Device pipeline per core (N-shard 25000 padded to 25088 = 196*128):
  - ACT: th = w0*p; sin1 = Sin(th); cos1 = Sin(pi/2 - |th|)  (Sin needs |arg|<=pi)
  - DVE: Chebyshev recurrence in fp16 (errors random-walk, don't amplify):
      t = (x_{m-1} * 2) . cos1   [scalar_tensor_tensor]
      x_m = t - x_{m-2}          [tensor_tensor]
    for x in {cos,sin}, m=2..16, over [128, 3*196] tiles.
  - PE:  per chunk c: mom[2,102] += Y2[:,2c:2c+2].T @ SC[:, :, c]  (fp16,
    1 cyc/col, f32 PSUM accumulate; lhsT col0 = valid mask kills padding).
    Split into two m-panels so PE overlaps the tail of the DVE recurrence.
"""

import os
from contextlib import ExitStack

import numpy as np

import concourse.bass as bass
import concourse.tile as tile
from concourse import mybir
from concourse.bass_utils import run_bass_kernel_spmd

N_CORES = 8
B = 256
D = 3
N_TOTAL = 200000
N_SHARD = N_TOTAL // N_CORES  # 25000
CHUNK = 128
N_CHUNKS = (N_SHARD + CHUNK - 1) // CHUNK  # 196
N_PAD = N_CHUNKS * CHUNK  # 25088

P_PERIOD = 12.0
M_FREQ = 17
W0 = 2.0 * np.pi / P_PERIOD
ROWS = 2 * M_FREQ * D  # 102 moment rows, idx = (2m+phi)*3+d, phi: 0=cos 1=sin
P_CLIP = 5.95  # |w0*p| <= 3.116 < pi

# panel split for DVE/PE overlap: rows [0, 54) need m<=8, rows [54, 102) m<=16
SPLIT_M = 9
SPLIT_ROW = 2 * SPLIT_M * D  # 54

_nc_cache = {}

# test.py introspection: last BassKernelResults from run_bass_kernel_spmd
LAST_RESULTS = None


def _build_nc():
    f32 = mybir.dt.float32
    f16 = mybir.dt.float16
    nc = bass.Bass(trn_type="TRN2")
    halfpi = float(np.pi / 2)
    _bias_t = nc.alloc_sbuf_tensor("const-float32-halfpi", [128, 1], f32)
    nc.gpsimd.memset(_bias_t.ap(), halfpi)
    nc.const_aps.aps[(f32, halfpi)] = _bias_t.ap()
    nc.all_engine_barrier()
    PT_d = nc.dram_tensor("PT", [CHUNK, D, N_CHUNKS], f32, kind="ExternalInput")
    Y2_d = nc.dram_tensor("Y2", [CHUNK, 2 * N_CHUNKS], f16, kind="ExternalInput")
    out_d = nc.dram_tensor("out", [2, ROWS], f32, kind="ExternalOutput")

    Alu = mybir.AluOpType
    Act = mybir.ActivationFunctionType

    with ExitStack() as ctx:
        tc = ctx.enter_context(tile.TileContext(nc))
        const = ctx.enter_context(tc.tile_pool(name="const", bufs=1))
        tpool = ctx.enter_context(tc.tile_pool(name="tpool", bufs=2))
        mpool = ctx.enter_context(tc.tile_pool(name="mpool", bufs=1, space="PSUM"))

        PT_t = const.tile([CHUNK, D, N_CHUNKS], f32)
        nc.gpsimd.dma_start(out=PT_t[:], in_=PT_d[:])
        Y2_t = const.tile([CHUNK, 2 * N_CHUNKS], f16)
        nc.gpsimd.dma_start(out=Y2_t[:], in_=Y2_d[:])

        SC_t = const.tile([CHUNK, ROWS, N_CHUNKS], f16)
        a_t = const.tile([CHUNK, D, N_CHUNKS], f32)

        def rows(m, phi):
            r = (2 * m + phi) * D
            return SC_t[:, r : r + D, :]

        # m=0: cos=1 (sum -> count / sum(y)), sin=0
        nc.vector.memset(rows(0, 0), 1.0)
        nc.vector.memset(rows(0, 1), 0.0)
        # m=1 base: sin1 = Sin(w0*p); cos1 = Sin(pi/2 - |w0*p|)
        nc.scalar.activation(rows(1, 1), PT_t[:], Act.Sin, scale=float(W0))
        nc.scalar.activation(a_t[:], PT_t[:], Act.Abs, scale=float(W0))
        nc.scalar.activation(
            rows(1, 0), a_t[:], Act.Sin, scale=-1.0, bias=float(np.pi / 2)
        )

        c1 = rows(1, 0)

        def emit_cheb(m):
            for phi in (0, 1):
                t = tpool.tile([CHUNK, D, N_CHUNKS], f16)
                nc.vector.scalar_tensor_tensor(
                    t[:], rows(m - 1, phi), 2.0, c1, Alu.mult, Alu.mult
                )
                nc.vector.tensor_tensor(
                    rows(m, phi), t[:], rows(m - 2, phi), Alu.subtract
                )

        def emit_panel(mom, r0, r1):
            for c in range(N_CHUNKS):
                nc.tensor.matmul(
                    mom[:, r0:r1],
                    lhsT=Y2_t[:, 2 * c : 2 * c + 2],
                    rhs=SC_t[:, r0:r1, c : c + 1],
                    start=(c == 0),
                    stop=(c == N_CHUNKS - 1),
                )

        mom = mpool.tile([2, ROWS], f32)
        for m in range(2, SPLIT_M):
            emit_cheb(m)
        emit_panel(mom, 0, SPLIT_ROW)
        for m in range(SPLIT_M, M_FREQ):
            emit_cheb(m)
        emit_panel(mom, SPLIT_ROW, ROWS)

        o_t = const.tile([2, ROWS], f32)
        nc.vector.tensor_copy(o_t[:], mom[:])
        nc.gpsimd.dma_start(out=out_d[:], in_=o_t[:])

    _strip_self_waits(nc)
    _split_multi_waits(nc)
    return nc


def _split_multi_waits(nc):
    """Walrus encodes at most one sync-wait per instruction on this target.

    Move all but the last wait of any multi-wait instruction onto preceding
    same-engine NoOps (in-order queues make sequential waiting equivalent to
    the ANDed wait set).
    """
    import bass_rust

    for bb_holder in nc.main_func.blocks:
        insts = list(bb_holder.instructions)
        out = []
        changed = False
        for i in insts:
            si = getattr(i, "sync_info", None)
            if (
                si is not None
                and len(si.on_wait) > 1
                and type(i).__name__ != "InstEventSemaphore"
            ):
                for w in si.on_wait[:-1]:
                    nop = mybir.InstNoOp(
                        name=nc.get_next_instruction_name(),
                        sync_info=bass_rust.SyncInfo(on_wait=[w], on_update=[]),
                        bass_nofuse=True,
                        engine=i.engine,
                    )
                    out.append(nop)
                i.sync_info = bass_rust.SyncInfo(
                    on_wait=[si.on_wait[-1]], on_update=list(si.on_update)
                )
                changed = True
            out.append(i)
        if changed:
            _replace_bb_instructions(bb_holder, out)


def _replace_bb_instructions(bb_holder, new_insts):
    bb = getattr(bb_holder, "bb", bb_holder)
    try:
        bb.instructions = new_insts
    except Exception:
        while len(bb.instructions):
            bb.instructions.pop()
        for x in new_insts:
            bb.add_instruction(x)


def _strip_self_waits(nc):
    """Drop semaphore waits that an in-order engine holds against itself.

    Tile emits WAW waits (e.g. temp-pool slot reuse) on the engine's own
    semaphore. In-order queues always satisfy these, but they push the
    per-instruction sync-wait count past what walrus codegen encodes.
    Only waits on semaphores updated exclusively by same-engine instructions
    are removed, and only for in-order engines (PE reorders LDWEIGHTS).
    """
    import bass_rust

    SAFE = (mybir.EngineType.Activation, mybir.EngineType.DVE, mybir.EngineType.Pool)
    insts = [i for bb in nc.main_func.blocks for i in bb.instructions]
    updaters = {}
    for i in insts:
        si = getattr(i, "sync_info", None)
        if si is None:
            continue
        for u in si.on_update:
            updaters.setdefault(u.id, set()).add(i.engine)
    for i in insts:
        if i.engine not in SAFE:
            continue
        si = getattr(i, "sync_info", None)
        if si is None or len(si.on_wait) <= 1:
            continue
        keep = [w for w in si.on_wait if updaters.get(w.id, {None}) != {i.engine}]
        if len(keep) != len(si.on_wait):
            i.sync_info = bass_rust.SyncInfo(
                on_wait=keep, on_update=list(si.on_update)
            )


def _get_nc():
    if "nc" not in _nc_cache:
        _nc_cache["nc"] = _build_nc()
    return _nc_cache["nc"]


def kernel(x, train_X, Y, W):
    global LAST_RESULTS
    x = np.ascontiguousarray(np.asarray(x, dtype=np.float32))
    train_X = np.ascontiguousarray(np.asarray(train_X, dtype=np.float32))
    Y = np.ascontiguousarray(np.asarray(Y, dtype=np.float32))
    W = np.ascontiguousarray(np.asarray(W, dtype=np.float32))

    xw = (x @ W.T).astype(np.float64)  # [B,3]
    proj = np.clip(train_X @ W.T, -P_CLIP, P_CLIP)  # [N,3] f32

    in_maps = []
    for s in range(N_CORES):
        pj = np.zeros((N_PAD, D), dtype=np.float32)
        pj[:N_SHARD] = proj[s * N_SHARD : (s + 1) * N_SHARD]
        # PT[p, d, c] = proj[c*128+p, d]
        PT = np.ascontiguousarray(pj.reshape(N_CHUNKS, CHUNK, D).transpose(1, 2, 0))

        y2 = np.zeros((N_PAD, 2), dtype=np.float16)
        y2[:N_SHARD, 0] = 1.0
        y2[:N_SHARD, 1] = Y[s * N_SHARD : (s + 1) * N_SHARD].astype(np.float16)
        # Y2[p, 2c+t] = y2[c*128+p, t]
        Y2 = np.ascontiguousarray(
            y2.reshape(N_CHUNKS, CHUNK, 2).transpose(1, 0, 2).reshape(CHUNK, -1)
        )
        in_maps.append({"PT": PT, "Y2": Y2})

    nc = _get_nc()
    res = run_bass_kernel_spmd(
        nc,
        in_maps,
        core_ids=list(range(N_CORES)),
        trace=bool(int(os.environ.get("KNN_TRACE", "0"))),
    )
    LAST_RESULTS = res

    mom = np.zeros((2, ROWS), dtype=np.float64)
    for r in res.results:
        mom += r["out"].astype(np.float64)
    # mom[w, (2m+phi)*3+d]
    mom = mom.reshape(2, M_FREQ, 2, D)  # [w, m, phi, d]

    m = np.arange(M_FREQ)
    om = m * W0
    a = (2.0 - (m == 0)) / P_PERIOD * np.sqrt(np.pi / 2.0) * np.exp(-(om**2) / 8.0)
    qarg = om[None, None, :] * xw[:, :, None]  # [B,3,M]
    qc = np.cos(qarg)
    qs = np.sin(qarg)
    S0 = np.einsum("m,bdm->bd", a, qc * mom[0, :, 0, :].T[None]) + np.einsum(
        "m,bdm->bd", a, qs * mom[0, :, 1, :].T[None]
    )
    S1 = np.einsum("m,bdm->bd", a, qc * mom[1, :, 0, :].T[None]) + np.einsum(
        "m,bdm->bd", a, qs * mom[1, :, 1, :].T[None]
    )
    return (S1 / S0).astype(np.float32)


# revision 7
# speedup vs baseline: 4.6267x; 1.0326x over previous
"""Gaussian-kernel (Nadaraya-Watson) regression on 8 TRN2 NeuronCores.

Reference: out[q,d] = sum_n Y[n]*K / sum_n K, K = exp(-2*(proj[n,d]-xw[q,d])^2),
proj = train_X @ W.T [N,3], xw = x @ W.T [B,3], N=200000, B=256, H=0.5.

Algorithm (Fourier / fast-Gauss): periodize the 1-D kernel with period P and
truncate its cosine series at M terms:
    exp(-2*D^2) ~= sum_m a_m cos(w_m D),  w_m = 2*pi*m/P
    cos(w_m (p-c)) = cos(w_m p)cos(w_m c) + sin(w_m p)sin(w_m c)
so each core only computes trig MOMENTS of its N-shard:
    mom[w, (m,phi,d)] = sum_n {1,y_n} * {cos,sin}(w_m * proj[n,d])
and the host combines the 8 partial moments and evaluates the tiny [B,3]
query-side sum in f64. P=12, M=17 -> rel err ~1e-3 (fp16 pipeline).

Device pipeline per core (N-shard 25000 padded to 25088 = 196*128):
  - ACT: th = w0*p; s1 = Sin(th); c1 = Sin(pi/2 - |th|)  (Sin needs |arg|<=pi)
  - DVE: scaled Chebyshev in fp16 with u_m = 2*cos(m th) (and sin partner):
      u_1 = base + base  (one tensor_tensor add)
      u_m = u_1 . u_{m-1} - u_{m-2}   (2 tensor_tensor per m, cos+sin pairs
      processed together on contiguous 6-row slices; tensor_tensor runs at
      the fp16 2x DVE rate, scalar_tensor_tensor would not)
  - PE:  per chunk c: mom[2,99] += Y2[:,2c:2c+2].T @ SC[:, :, c]  (fp16,
    f32 PSUM accumulate; lhsT col0 = valid mask kills padding).
  The 196 columns are processed in KNN_CB blocks: PE consumes block b while
  DVE runs the recurrence on block b+1.
Host divides all moments by 2 (u-scaling) and corrects nothing else.
"""

import os
from contextlib import ExitStack

import numpy as np

import concourse.bass as bass
import concourse.tile as tile
from concourse import mybir
from concourse.bass_utils import run_bass_kernel_spmd

N_CORES = 8
B = 256
D = 3
N_TOTAL = 200000
N_SHARD = N_TOTAL // N_CORES  # 25000
CHUNK = 128
N_CHUNKS = (N_SHARD + CHUNK - 1) // CHUNK  # 196
N_PAD = N_CHUNKS * CHUNK  # 25088

P_PERIOD = 12.0
M_FREQ = 17
W0 = 2.0 * np.pi / P_PERIOD
# rows: 0..2 = m0-cos (memset 2.0); 3+(m-1)*6+phi*3+d for m>=1, phi 0=cos 1=sin
ROWS = 3 + (M_FREQ - 1) * 6  # 99
P_CLIP = 5.95  # |w0*p| <= 3.116 < pi

CB = int(os.environ.get("KNN_CB", "3"))

_nc_cache = {}
LAST_RESULTS = None


def _blocks():
    edges = [round(i * N_CHUNKS / CB) for i in range(CB + 1)]
    return list(zip(edges[:-1], edges[1:]))


def _build_nc():
    f32 = mybir.dt.float32
    f16 = mybir.dt.float16
    nc = bass.Bass(trn_type="TRN2")
    halfpi = float(np.pi / 2)
    _bias_t = nc.alloc_sbuf_tensor("const-float32-halfpi", [128, 1], f32)
    nc.gpsimd.memset(_bias_t.ap(), halfpi)
    nc.const_aps.aps[(f32, halfpi)] = _bias_t.ap()
    nc.all_engine_barrier()

    PT_d = nc.dram_tensor("PT", [CHUNK, D, N_CHUNKS], f32, kind="ExternalInput")
    Y2_d = nc.dram_tensor("Y2", [CHUNK, 2 * N_CHUNKS], f16, kind="ExternalInput")
    out_d = nc.dram_tensor("out", [2, ROWS], f32, kind="ExternalOutput")

    Alu = mybir.AluOpType
    Act = mybir.ActivationFunctionType

    with ExitStack() as ctx:
        tc = ctx.enter_context(tile.TileContext(nc))
        const = ctx.enter_context(tc.tile_pool(name="const", bufs=1))
        tpool = ctx.enter_context(tc.tile_pool(name="tpool", bufs=2))
        mpool = ctx.enter_context(tc.tile_pool(name="mpool", bufs=1, space="PSUM"))

        # ACT table prefetch: a 1-col Sin with no data deps runs during DMA
        warm = const.tile([CHUNK, 1], f32)
        nc.scalar.activation(warm[:], _bias_t.ap(), Act.Sin)

        PT_t = const.tile([CHUNK, D, N_CHUNKS], f32)
        nc.gpsimd.dma_start(out=PT_t[:], in_=PT_d[:])
        Y2_t = const.tile([CHUNK, 2 * N_CHUNKS], f16)
        nc.scalar.dma_start(out=Y2_t[:], in_=Y2_d[:])

        SC_t = const.tile([CHUNK, ROWS, N_CHUNKS], f16)
        B6_t = const.tile([CHUNK, 6, N_CHUNKS], f16)  # [c1(3) | s1(3)] base
        B6R_t = const.tile([CHUNK, 6, N_CHUNKS], f16)  # [c1(3) | c1(3)]
        U1R_t = const.tile([CHUNK, 6, N_CHUNKS], f16)  # [2c1(3) | 2c1(3)]
        U0_t = const.tile([CHUNK, 6, N_CHUNKS], f16)  # [2,2,2 | 0,0,0]
        a_t = const.tile([CHUNK, D, N_CHUNKS], f32)

        nc.gpsimd.memset(SC_t[:, 0:3, :], 2.0)
        nc.gpsimd.memset(U0_t[:, 0:3, :], 2.0)
        nc.gpsimd.memset(U0_t[:, 3:6, :], 0.0)

        def pair(m):
            r = 3 + (m - 1) * 6
            return SC_t[:, r : r + 6, :]

        # base: c1 = Sin(pi/2 - |w0 p|), s1 = Sin(w0 p)
        nc.scalar.activation(a_t[:], PT_t[:], Act.Abs, scale=float(W0))
        nc.scalar.activation(B6_t[:, 0:3, :], a_t[:], Act.Sin, scale=-1.0, bias=halfpi)
        nc.scalar.activation(B6_t[:, 3:6, :], PT_t[:], Act.Sin, scale=float(W0))
        nc.scalar.activation(
            B6R_t[:, 0:3, :], a_t[:], Act.Sin, scale=-1.0, bias=halfpi
        )
        nc.scalar.activation(
            B6R_t[:, 3:6, :], a_t[:], Act.Sin, scale=-1.0, bias=halfpi
        )
        # u1 moment rows = [2c1|2s1]; chain multiplier = [2c1|2c1]
        nc.vector.tensor_tensor(pair(1), B6_t[:], B6_t[:], Alu.add)
        nc.vector.tensor_tensor(U1R_t[:], B6R_t[:], B6R_t[:], Alu.add)

        mom = mpool.tile([2, ROWS], f32)

        for c0, c1 in _blocks():
            for m in range(2, M_FREQ):
                prevprev = U0_t[:, :, c0:c1] if m == 2 else pair(m - 2)[:, :, c0:c1]
                t = tpool.tile([CHUNK, 6, N_CHUNKS], f16)
                nc.vector.tensor_tensor(
                    t[:, :, c0:c1], pair(m - 1)[:, :, c0:c1],
                    U1R_t[:, :, c0:c1], Alu.mult,
                )
                nc.vector.tensor_tensor(
                    pair(m)[:, :, c0:c1], t[:, :, c0:c1], prevprev, Alu.subtract
                )
            for c in range(c0, c1):
                nc.tensor.matmul(
                    mom[:],
                    lhsT=Y2_t[:, 2 * c : 2 * c + 2],
                    rhs=SC_t[:, :, c],
                    start=(c == 0),
                    stop=(c == N_CHUNKS - 1),
                )

        o_t = const.tile([2, ROWS], f32)
        nc.vector.tensor_copy(o_t[:], mom[:])
        nc.gpsimd.dma_start(out=out_d[:], in_=o_t[:])

    _strip_self_waits(nc)
    _split_multi_waits(nc)
    return nc


def _split_multi_waits(nc):
    """Walrus encodes at most one sync-wait per instruction on this target.

    Move all but the last wait of any multi-wait instruction onto preceding
    same-engine NoOps (in-order queues make sequential waiting equivalent to
    the ANDed wait set).
    """
    import bass_rust

    for bb_holder in nc.main_func.blocks:
        insts = list(bb_holder.instructions)
        out = []
        changed = False
        for i in insts:
            si = getattr(i, "sync_info", None)
            if (
                si is not None
                and len(si.on_wait) > 1
                and type(i).__name__ != "InstEventSemaphore"
            ):
                for w in si.on_wait[:-1]:
                    nop = mybir.InstNoOp(
                        name=nc.get_next_instruction_name(),
                        sync_info=bass_rust.SyncInfo(on_wait=[w], on_update=[]),
                        bass_nofuse=True,
                        engine=i.engine,
                    )
                    out.append(nop)
                i.sync_info = bass_rust.SyncInfo(
                    on_wait=[si.on_wait[-1]], on_update=list(si.on_update)
                )
                changed = True
            out.append(i)
        if changed:
            _replace_bb_instructions(bb_holder, out)


def _replace_bb_instructions(bb_holder, new_insts):
    bb = getattr(bb_holder, "bb", bb_holder)
    try:
        bb.instructions = new_insts
    except Exception:
        while len(bb.instructions):
            bb.instructions.pop()
        for x in new_insts:
            bb.add_instruction(x)


def _strip_self_waits(nc):
    """Drop semaphore waits that an in-order engine holds against itself.

    Tile emits WAW waits (e.g. temp-pool slot reuse) on the engine's own
    semaphore. In-order queues always satisfy these, but they push the
    per-instruction sync-wait count past what walrus codegen encodes.
    Only waits on semaphores updated exclusively by same-engine instructions
    are removed, and only for in-order engines (PE reorders LDWEIGHTS).
    """
    import bass_rust

    SAFE = (mybir.EngineType.Activation, mybir.EngineType.DVE, mybir.EngineType.Pool)
    insts = [i for bb in nc.main_func.blocks for i in bb.instructions]
    updaters = {}
    for i in insts:
        si = getattr(i, "sync_info", None)
        if si is None:
            continue
        for u in si.on_update:
            updaters.setdefault(u.id, set()).add(i.engine)
    for i in insts:
        if i.engine not in SAFE:
            continue
        si = getattr(i, "sync_info", None)
        if si is None or len(si.on_wait) <= 1:
            continue
        keep = [w for w in si.on_wait if updaters.get(w.id, {None}) != {i.engine}]
        if len(keep) != len(si.on_wait):
            i.sync_info = bass_rust.SyncInfo(
                on_wait=keep, on_update=list(si.on_update)
            )


def _get_nc():
    if "nc" not in _nc_cache:
        _nc_cache["nc"] = _build_nc()
    return _nc_cache["nc"]


def kernel(x, train_X, Y, W):
    global LAST_RESULTS
    x = np.ascontiguousarray(np.asarray(x, dtype=np.float32))
    train_X = np.ascontiguousarray(np.asarray(train_X, dtype=np.float32))
    Y = np.ascontiguousarray(np.asarray(Y, dtype=np.float32))
    W = np.ascontiguousarray(np.asarray(W, dtype=np.float32))

    xw = (x @ W.T).astype(np.float64)  # [B,3]
    proj = np.clip(train_X @ W.T, -P_CLIP, P_CLIP)  # [N,3] f32

    in_maps = []
    for s in range(N_CORES):
        pj = np.zeros((N_PAD, D), dtype=np.float32)
        pj[:N_SHARD] = proj[s * N_SHARD : (s + 1) * N_SHARD]
        PT = np.ascontiguousarray(pj.reshape(N_CHUNKS, CHUNK, D).transpose(1, 2, 0))

        y2 = np.zeros((N_PAD, 2), dtype=np.float16)
        y2[:N_SHARD, 0] = 1.0
        y2[:N_SHARD, 1] = Y[s * N_SHARD : (s + 1) * N_SHARD].astype(np.float16)
        Y2 = np.ascontiguousarray(
            y2.reshape(N_CHUNKS, CHUNK, 2).transpose(1, 0, 2).reshape(CHUNK, -1)
        )
        in_maps.append({"PT": PT, "Y2": Y2})

    nc = _get_nc()
    res = run_bass_kernel_spmd(
        nc,
        in_maps,
        core_ids=list(range(N_CORES)),
        trace=bool(int(os.environ.get("KNN_TRACE", "0"))),
    )
    LAST_RESULTS = res

    raw = np.zeros((2, ROWS), dtype=np.float64)
    for r in res.results:
        raw += r["out"].astype(np.float64)
    raw *= 0.5  # u-scaling: device rows are 2*cos / 2*sin (m0 memset 2.0)

    # unpack rows -> mom[w, m, phi, d]
    mom = np.zeros((2, M_FREQ, 2, D), dtype=np.float64)
    mom[:, 0, 0, :] = raw[:, 0:3]
    body = raw[:, 3:].reshape(2, M_FREQ - 1, 2, D)
    mom[:, 1:, :, :] = body

    m = np.arange(M_FREQ)
    om = m * W0
    a = (2.0 - (m == 0)) / P_PERIOD * np.sqrt(np.pi / 2.0) * np.exp(-(om**2) / 8.0)
    qarg = om[None, None, :] * xw[:, :, None]  # [B,3,M]
    qc = np.cos(qarg)
    qs = np.sin(qarg)
    S0 = np.einsum("m,bdm->bd", a, qc * mom[0, :, 0, :].T[None]) + np.einsum(
        "m,bdm->bd", a, qs * mom[0, :, 1, :].T[None]
    )
    S1 = np.einsum("m,bdm->bd", a, qc * mom[1, :, 0, :].T[None]) + np.einsum(
        "m,bdm->bd", a, qs * mom[1, :, 1, :].T[None]
    )
    return (S1 / S0).astype(np.float32)


# revision 8
# speedup vs baseline: 5.2886x; 1.1431x over previous
"""Gaussian-kernel (Nadaraya-Watson) regression on 8 TRN2 NeuronCores.

Reference: out[q,d] = sum_n Y[n]*K / sum_n K, K = exp(-2*(proj[n,d]-xw[q,d])^2),
proj = train_X @ W.T [N,3], xw = x @ W.T [B,3], N=200000, B=256, H=0.5.

Algorithm (Fourier / fast-Gauss): periodize the 1-D kernel with period P and
truncate its cosine series at M terms:
    exp(-2*D^2) ~= sum_m a_m cos(w_m D),  w_m = 2*pi*m/P
    cos(w_m (p-c)) = cos(w_m p)cos(w_m c) + sin(w_m p)sin(w_m c)
so each core only computes trig MOMENTS of its N-shard:
    mom[w, (m,phi,d)] = sum_n {1,y_n} * {cos,sin}(w_m * proj[n,d])
and the host combines the 8 partial moments and evaluates the tiny [B,3]
query-side sum in f64. P=12, M=17 -> rel err ~1e-3 (fp16 pipeline).

Device pipeline per core (N-shard 25000 padded to 25088 = 196*128).
All SBUF value tiles are chunk-major [128, 196, rows] so the PE's per-chunk
rhs SC[:, c, :] is CONTIGUOUS (a strided rhs streams ~4x slower):
  - host sends TH = [pi/2-|th| (3) | th (3)] with th = w0*clip(p), so ONE
    ACT Sin pass yields B6 = [cos1(3) | sin1(3)]  (Sin needs |arg|<=pi)
  - DVE: scaled Chebyshev in fp16, u_m = 2*cos(m th) (+ sin partner):
      pair(1) = B6 + B6; U1R = [2c1|2c1] (built on GpSimd)
      t = pair(m-1) . U1R ; pair(m) = t - pair(m-2)   (2 full-width
      tensor_tensor per m; DVE per-instr overhead ~160ns so fewer/bigger ops)
  - PE: per chunk c: mom[2, r0:r1] += Y2[:,2c:2c+2].T @ SC[:, c, r0:r1]
    (fp16, f32 PSUM accumulate; lhsT col0 = valid mask kills padding), in two
    m-panels so PE overlaps the tail of the recurrence (LDW hides under MM).
Host divides all moments by 2 (u-scaling).
"""

import os
from contextlib import ExitStack

import numpy as np

import concourse.bass as bass
import concourse.tile as tile
from concourse import mybir
from concourse.bass_utils import run_bass_kernel_spmd

N_CORES = 8
B = 256
D = 3
N_TOTAL = 200000
N_SHARD = N_TOTAL // N_CORES  # 25000
CHUNK = 128
N_CHUNKS = (N_SHARD + CHUNK - 1) // CHUNK  # 196
N_PAD = N_CHUNKS * CHUNK  # 25088

P_PERIOD = 12.0
M_FREQ = 17
W0 = 2.0 * np.pi / P_PERIOD
# rows: 0..2 = m0-cos (memset 2.0); 3+(m-1)*6+phi*3+d for m>=1, phi 0=cos 1=sin
ROWS = 3 + (M_FREQ - 1) * 6  # 99
P_CLIP = 5.95  # |w0*p| <= 3.116 < pi

SPLIT_M = int(os.environ.get("KNN_SPLIT_M", "10"))  # panel A covers m < SPLIT_M
SPLIT_ROW = 3 + (SPLIT_M - 1) * 6

_nc_cache = {}
LAST_RESULTS = None


def _build_nc():
    f32 = mybir.dt.float32
    f16 = mybir.dt.float16
    nc = bass.Bass(trn_type="TRN2")

    TH_d = nc.dram_tensor("TH", [CHUNK, N_CHUNKS, 6], f32, kind="ExternalInput")
    Y2_d = nc.dram_tensor("Y2", [CHUNK, 2 * N_CHUNKS], f16, kind="ExternalInput")
    out_d = nc.dram_tensor("out", [2, ROWS], f32, kind="ExternalOutput")

    Alu = mybir.AluOpType
    Act = mybir.ActivationFunctionType
    HALF = N_CHUNKS // 2  # 98

    with ExitStack() as ctx:
        tc = ctx.enter_context(tile.TileContext(nc))
        const = ctx.enter_context(tc.tile_pool(name="const", bufs=1))
        tpool = ctx.enter_context(tc.tile_pool(name="tpool", bufs=2))
        mpool = ctx.enter_context(tc.tile_pool(name="mpool", bufs=1, space="PSUM"))

        TH_t = const.tile([CHUNK, N_CHUNKS, 6], f32)
        # split the input DMA so the first ACT starts after half the bytes
        nc.gpsimd.dma_start(out=TH_t[:, 0:HALF, :], in_=TH_d[:, 0:HALF, :])
        nc.gpsimd.dma_start(
            out=TH_t[:, HALF:N_CHUNKS, :], in_=TH_d[:, HALF:N_CHUNKS, :]
        )
        Y2_t = const.tile([CHUNK, 2 * N_CHUNKS], f16)
        nc.scalar.dma_start(out=Y2_t[:], in_=Y2_d[:])

        SC_t = const.tile([CHUNK, N_CHUNKS, ROWS], f16)
        B6_t = const.tile([CHUNK, N_CHUNKS, 6], f16)  # [cos1(3) | sin1(3)]
        U1R_t = const.tile([CHUNK, N_CHUNKS, 6], f16)  # [2c1(3) | 2c1(3)]
        U0_t = const.tile([CHUNK, N_CHUNKS, 6], f16)  # [2,2,2 | 0,0,0]

        nc.gpsimd.memset(SC_t[:, :, 0:3], 2.0)
        nc.gpsimd.memset(U0_t[:, :, 0:3], 2.0)
        nc.gpsimd.memset(U0_t[:, :, 3:6], 0.0)

        def pair(m):
            r = 3 + (m - 1) * 6
            return SC_t[:, :, r : r + 6]

        # base: B6 = Sin(TH) = [cos1 | sin1], one func, two half-width passes
        nc.scalar.activation(B6_t[:, 0:HALF, :], TH_t[:, 0:HALF, :], Act.Sin)
        nc.scalar.activation(
            B6_t[:, HALF:N_CHUNKS, :], TH_t[:, HALF:N_CHUNKS, :], Act.Sin
        )

        # pair(1) = [2c1|2s1] on DVE; U1R = [2c1|2c1] on GpSimd (keeps DVE free)
        nc.vector.tensor_tensor(pair(1), B6_t[:], B6_t[:], Alu.add)
        nc.gpsimd.tensor_tensor(
            U1R_t[:, :, 0:3], B6_t[:, :, 0:3], B6_t[:, :, 0:3], Alu.add
        )
        nc.gpsimd.tensor_tensor(
            U1R_t[:, :, 3:6], B6_t[:, :, 0:3], B6_t[:, :, 0:3], Alu.add
        )

        def emit_cheb(m):
            prevprev = U0_t[:] if m == 2 else pair(m - 2)
            t = tpool.tile([CHUNK, N_CHUNKS, 6], f16)
            nc.vector.tensor_tensor(t[:], pair(m - 1), U1R_t[:], Alu.mult)
            nc.vector.tensor_tensor(pair(m), t[:], prevprev, Alu.subtract)

        def emit_panel(mom, r0, r1):
            for c in range(N_CHUNKS):
                nc.tensor.matmul(
                    mom[:, r0:r1],
                    lhsT=Y2_t[:, 2 * c : 2 * c + 2],
                    rhs=SC_t[:, c, r0:r1],
                    start=(c == 0),
                    stop=(c == N_CHUNKS - 1),
                )

        mom = mpool.tile([2, ROWS], f32)
        for m in range(2, SPLIT_M):
            emit_cheb(m)
        emit_panel(mom, 0, SPLIT_ROW)
        for m in range(SPLIT_M, M_FREQ):
            emit_cheb(m)
        emit_panel(mom, SPLIT_ROW, ROWS)

        o_t = const.tile([2, ROWS], f32)
        nc.vector.tensor_copy(o_t[:], mom[:])
        nc.gpsimd.dma_start(out=out_d[:], in_=o_t[:])

    _strip_self_waits(nc)
    _split_multi_waits(nc)
    return nc


def _split_multi_waits(nc):
    """Walrus encodes at most one sync-wait per instruction on this target.

    Move all but the last wait of any multi-wait instruction onto preceding
    same-engine NoOps (in-order queues make sequential waiting equivalent to
    the ANDed wait set).
    """
    import bass_rust

    for bb_holder in nc.main_func.blocks:
        insts = list(bb_holder.instructions)
        out = []
        changed = False
        for i in insts:
            si = getattr(i, "sync_info", None)
            if (
                si is not None
                and len(si.on_wait) > 1
                and type(i).__name__ != "InstEventSemaphore"
            ):
                for w in si.on_wait[:-1]:
                    nop = mybir.InstNoOp(
                        name=nc.get_next_instruction_name(),
                        sync_info=bass_rust.SyncInfo(on_wait=[w], on_update=[]),
                        bass_nofuse=True,
                        engine=i.engine,
                    )
                    out.append(nop)
                i.sync_info = bass_rust.SyncInfo(
                    on_wait=[si.on_wait[-1]], on_update=list(si.on_update)
                )
                changed = True
            out.append(i)
        if changed:
            _replace_bb_instructions(bb_holder, out)


def _replace_bb_instructions(bb_holder, new_insts):
    bb = getattr(bb_holder, "bb", bb_holder)
    try:
        bb.instructions = new_insts
    except Exception:
        while len(bb.instructions):
            bb.instructions.pop()
        for x in new_insts:
            bb.add_instruction(x)


def _strip_self_waits(nc):
    """Drop semaphore waits that an in-order engine holds against itself.

    Tile emits WAW waits (e.g. temp-pool slot reuse) on the engine's own
    semaphore. In-order queues always satisfy these, but they push the
    per-instruction sync-wait count past what walrus codegen encodes.
    Only waits on semaphores updated exclusively by same-engine instructions
    are removed, and only for in-order engines (PE reorders LDWEIGHTS).
    """
    import bass_rust

    SAFE = (mybir.EngineType.Activation, mybir.EngineType.DVE, mybir.EngineType.Pool)
    insts = [i for bb in nc.main_func.blocks for i in bb.instructions]
    updaters = {}
    for i in insts:
        si = getattr(i, "sync_info", None)
        if si is None:
            continue
        for u in si.on_update:
            updaters.setdefault(u.id, set()).add(i.engine)
    for i in insts:
        if i.engine not in SAFE:
            continue
        si = getattr(i, "sync_info", None)
        if si is None or len(si.on_wait) <= 1:
            continue
        keep = [w for w in si.on_wait if updaters.get(w.id, {None}) != {i.engine}]
        if len(keep) != len(si.on_wait):
            i.sync_info = bass_rust.SyncInfo(
                on_wait=keep, on_update=list(si.on_update)
            )


def _get_nc():
    if "nc" not in _nc_cache:
        _nc_cache["nc"] = _build_nc()
    return _nc_cache["nc"]


def kernel(x, train_X, Y, W):
    global LAST_RESULTS
    x = np.ascontiguousarray(np.asarray(x, dtype=np.float32))
    train_X = np.ascontiguousarray(np.asarray(train_X, dtype=np.float32))
    Y = np.ascontiguousarray(np.asarray(Y, dtype=np.float32))
    W = np.ascontiguousarray(np.asarray(W, dtype=np.float32))

    xw = (x @ W.T).astype(np.float64)  # [B,3]
    th = (W0 * np.clip(train_X @ W.T, -P_CLIP, P_CLIP)).astype(np.float32)
    thc = (np.pi / 2 - np.abs(th)).astype(np.float32)

    in_maps = []
    for s in range(N_CORES):
        th6 = np.zeros((N_PAD, 6), dtype=np.float32)
        th6[:N_SHARD, 0:3] = thc[s * N_SHARD : (s + 1) * N_SHARD]
        th6[N_SHARD:, 0:3] = np.pi / 2  # pad: cos arg -> sin(pi/2)=1 (masked)
        th6[:N_SHARD, 3:6] = th[s * N_SHARD : (s + 1) * N_SHARD]
        # TH[p, c, :] = th6[c*128+p, :]
        TH = np.ascontiguousarray(th6.reshape(N_CHUNKS, CHUNK, 6).transpose(1, 0, 2))

        y2 = np.zeros((N_PAD, 2), dtype=np.float16)
        y2[:N_SHARD, 0] = 1.0
        y2[:N_SHARD, 1] = Y[s * N_SHARD : (s + 1) * N_SHARD].astype(np.float16)
        Y2 = np.ascontiguousarray(
            y2.reshape(N_CHUNKS, CHUNK, 2).transpose(1, 0, 2).reshape(CHUNK, -1)
        )
        in_maps.append({"TH": TH, "Y2": Y2})

    nc = _get_nc()
    res = run_bass_kernel_spmd(
        nc,
        in_maps,
        core_ids=list(range(N_CORES)),
        trace=bool(int(os.environ.get("KNN_TRACE", "0"))),
    )
    LAST_RESULTS = res

    raw = np.zeros((2, ROWS), dtype=np.float64)
    for r in res.results:
        raw += r["out"].astype(np.float64)
    raw *= 0.5  # u-scaling: device rows are 2*cos / 2*sin (m0 memset 2.0)

    mom = np.zeros((2, M_FREQ, 2, D), dtype=np.float64)
    mom[:, 0, 0, :] = raw[:, 0:3]
    mom[:, 1:, :, :] = raw[:, 3:].reshape(2, M_FREQ - 1, 2, D)

    m = np.arange(M_FREQ)
    om = m * W0
    a = (2.0 - (m == 0)) / P_PERIOD * np.sqrt(np.pi / 2.0) * np.exp(-(om**2) / 8.0)
    qarg = om[None, None, :] * xw[:, :, None]  # [B,3,M]
    qc = np.cos(qarg)
    qs = np.sin(qarg)
    S0 = np.einsum("m,bdm->bd", a, qc * mom[0, :, 0, :].T[None]) + np.einsum(
        "m,bdm->bd", a, qs * mom[0, :, 1, :].T[None]
    )
    S1 = np.einsum("m,bdm->bd", a, qc * mom[1, :, 0, :].T[None]) + np.einsum(
        "m,bdm->bd", a, qs * mom[1, :, 1, :].T[None]
    )
    return (S1 / S0).astype(np.float32)


# revision 9
# speedup vs baseline: 6.1101x; 1.1553x over previous
"""Gaussian-kernel (Nadaraya-Watson) regression on 8 TRN2 NeuronCores.

Reference: out[q,d] = sum_n Y[n]*K / sum_n K, K = exp(-2*(proj[n,d]-xw[q,d])^2),
proj = train_X @ W.T [N,3], xw = x @ W.T [B,3], N=200000, B=256, H=0.5.

Algorithm (Fourier / fast-Gauss): periodize the 1-D kernel with period P and
truncate its cosine series at M terms:
    exp(-2*D^2) ~= sum_m a_m cos(w_m D),  w_m = 2*pi*m/P
    cos(w_m (p-c)) = cos(w_m p)cos(w_m c) + sin(w_m p)sin(w_m c)
so each core only computes trig MOMENTS of its N-shard:
    mom[w, (m,phi,d)] = sum_n {1,y_n} * {cos,sin}(w_m * proj[n,d])
and the host combines the 8 partial moments and evaluates the tiny [B,3]
query-side sum in f64. P=12, M=17 -> rel err ~1e-3 (fp16 pipeline).

Device pipeline per core (N-shard 25000 padded to 25088 = 196*128).
SC is m-major [128, 17, 196, 6]: each m-group SC[:,m] is one CONTIGUOUS
[128, 1176] block (DVE fp16 tensor_tensor needs long packed runs for its 2x
rate), while the PE rhs SC[:, :, c, :] reads 12-byte runs (vs 2-byte with a
rows-major layout, which streams ~4x slower):
  - host sends TH = [pi/2-|th| (3) | th (3)], THR = [pi/2-|th|]*2, th =
    w0*clip(p); ONE ACT Sin pass each: B6 = [cos1|sin1], B6R = [cos1|cos1]
  - DVE: scaled Chebyshev in fp16, u_m = 2*cos(m th) (+ sin partner):
      SC[:,1] = B6 + B6; U1R = B6R + B6R; SC[:,0] = memset [2,2,2,0,0,0]
      t = SC[:,m-1] . U1R ; SC[:,m] = t - SC[:,m-2]
  - PE: per chunk c: mom[2, cols] += Y2[:,2c:2c+2].T @ SC[:, m-range, c, :]
    (fp16, f32 PSUM accumulate; lhsT col0 = valid mask kills padding), in two
    m-panels so PE overlaps the tail of the recurrence.
Host divides all moments by 2 (u-scaling).
"""

import os
from contextlib import ExitStack

import numpy as np

import concourse.bass as bass
import concourse.tile as tile
from concourse import mybir
from concourse.bass_utils import run_bass_kernel_spmd

N_CORES = 8
B = 256
D = 3
N_TOTAL = 200000
N_SHARD = N_TOTAL // N_CORES  # 25000
CHUNK = 128
N_CHUNKS = (N_SHARD + CHUNK - 1) // CHUNK  # 196
N_PAD = N_CHUNKS * CHUNK  # 25088

P_PERIOD = 12.0
M_FREQ = 17
W0 = 2.0 * np.pi / P_PERIOD
ROWS = 6 * M_FREQ  # 102; row = m*6 + phi*3 + d
P_CLIP = 5.95  # |w0*p| <= 3.116 < pi

SPLIT_M = int(os.environ.get("KNN_SPLIT_M", "10"))  # panel A covers m < SPLIT_M

_nc_cache = {}
LAST_RESULTS = None


def _build_nc():
    f32 = mybir.dt.float32
    f16 = mybir.dt.float16
    nc = bass.Bass(trn_type="TRN2")

    TH_d = nc.dram_tensor("TH", [CHUNK, N_CHUNKS, 6], f32, kind="ExternalInput")
    THR_d = nc.dram_tensor("THR", [CHUNK, N_CHUNKS, 6], f32, kind="ExternalInput")
    Y2_d = nc.dram_tensor("Y2", [CHUNK, 2 * N_CHUNKS], f16, kind="ExternalInput")
    out_d = nc.dram_tensor("out", [2, ROWS], f32, kind="ExternalOutput")

    Alu = mybir.AluOpType
    Act = mybir.ActivationFunctionType
    HALF = N_CHUNKS // 2  # 98

    with ExitStack() as ctx:
        tc = ctx.enter_context(tile.TileContext(nc))
        const = ctx.enter_context(tc.tile_pool(name="const", bufs=1))
        tpool = ctx.enter_context(tc.tile_pool(name="tpool", bufs=2))
        mpool = ctx.enter_context(tc.tile_pool(name="mpool", bufs=1, space="PSUM"))

        TH_t = const.tile([CHUNK, N_CHUNKS, 6], f32)
        nc.gpsimd.dma_start(out=TH_t[:, 0:HALF, :], in_=TH_d[:, 0:HALF, :])
        nc.gpsimd.dma_start(
            out=TH_t[:, HALF:N_CHUNKS, :], in_=TH_d[:, HALF:N_CHUNKS, :]
        )
        THR_t = const.tile([CHUNK, N_CHUNKS, 6], f32)
        nc.gpsimd.dma_start(out=THR_t[:], in_=THR_d[:])
        Y2_t = const.tile([CHUNK, 2 * N_CHUNKS], f16)
        nc.scalar.dma_start(out=Y2_t[:], in_=Y2_d[:])

        SC_t = const.tile([CHUNK, M_FREQ, N_CHUNKS, 6], f16)
        B6_t = const.tile([CHUNK, N_CHUNKS, 6], f16)  # [cos1(3) | sin1(3)]
        B6R_t = const.tile([CHUNK, N_CHUNKS, 6], f16)  # [cos1(3) | cos1(3)]
        U1R_t = const.tile([CHUNK, N_CHUNKS, 6], f16)  # [2c1(3) | 2c1(3)]

        # m0 group doubles as u_0 = [2,2,2,0,0,0] and yields count / sum(y)
        nc.gpsimd.memset(SC_t[:, 0, :, 0:3], 2.0)
        nc.gpsimd.memset(SC_t[:, 0, :, 3:6], 0.0)

        # base: one Sin pass per input (args within +-pi by construction)
        nc.scalar.activation(B6_t[:, 0:HALF, :], TH_t[:, 0:HALF, :], Act.Sin)
        nc.scalar.activation(
            B6_t[:, HALF:N_CHUNKS, :], TH_t[:, HALF:N_CHUNKS, :], Act.Sin
        )
        nc.scalar.activation(B6R_t[:], THR_t[:], Act.Sin)

        nc.vector.tensor_tensor(SC_t[:, 1], B6_t[:], B6_t[:], Alu.add)
        nc.vector.tensor_tensor(U1R_t[:], B6R_t[:], B6R_t[:], Alu.add)

        def emit_cheb(m):
            t = tpool.tile([CHUNK, N_CHUNKS, 6], f16)
            nc.vector.tensor_tensor(t[:], SC_t[:, m - 1], U1R_t[:], Alu.mult)
            nc.vector.tensor_tensor(SC_t[:, m], t[:], SC_t[:, m - 2], Alu.subtract)

        def emit_panel(mom, m0, m1):
            for c in range(N_CHUNKS):
                nc.tensor.matmul(
                    mom[:, m0 * 6 : m1 * 6],
                    lhsT=Y2_t[:, 2 * c : 2 * c + 2],
                    rhs=SC_t[:, m0:m1, c, :],
                    start=(c == 0),
                    stop=(c == N_CHUNKS - 1),
                )

        mom = mpool.tile([2, ROWS], f32)
        for m in range(2, SPLIT_M):
            emit_cheb(m)
        emit_panel(mom, 0, SPLIT_M)
        for m in range(SPLIT_M, M_FREQ):
            emit_cheb(m)
        emit_panel(mom, SPLIT_M, M_FREQ)

        o_t = const.tile([2, ROWS], f32)
        nc.vector.tensor_copy(o_t[:], mom[:])
        nc.gpsimd.dma_start(out=out_d[:], in_=o_t[:])

    _strip_self_waits(nc)
    _split_multi_waits(nc)
    return nc


def _split_multi_waits(nc):
    """Walrus encodes at most one sync-wait per instruction on this target.

    Move all but the last wait of any multi-wait instruction onto preceding
    same-engine NoOps (in-order queues make sequential waiting equivalent to
    the ANDed wait set).
    """
    import bass_rust

    for bb_holder in nc.main_func.blocks:
        insts = list(bb_holder.instructions)
        out = []
        changed = False
        for i in insts:
            si = getattr(i, "sync_info", None)
            if (
                si is not None
                and len(si.on_wait) > 1
                and type(i).__name__ != "InstEventSemaphore"
            ):
                for w in si.on_wait[:-1]:
                    nop = mybir.InstNoOp(
                        name=nc.get_next_instruction_name(),
                        sync_info=bass_rust.SyncInfo(on_wait=[w], on_update=[]),
                        bass_nofuse=True,
                        engine=i.engine,
                    )
                    out.append(nop)
                i.sync_info = bass_rust.SyncInfo(
                    on_wait=[si.on_wait[-1]], on_update=list(si.on_update)
                )
                changed = True
            out.append(i)
        if changed:
            _replace_bb_instructions(bb_holder, out)


def _replace_bb_instructions(bb_holder, new_insts):
    bb = getattr(bb_holder, "bb", bb_holder)
    try:
        bb.instructions = new_insts
    except Exception:
        while len(bb.instructions):
            bb.instructions.pop()
        for x in new_insts:
            bb.add_instruction(x)


def _strip_self_waits(nc):
    """Drop semaphore waits that an in-order engine holds against itself.

    Tile emits WAW waits (e.g. temp-pool slot reuse) on the engine's own
    semaphore. In-order queues always satisfy these, but they push the
    per-instruction sync-wait count past what walrus codegen encodes.
    Only waits on semaphores updated exclusively by same-engine instructions
    are removed, and only for in-order engines (PE reorders LDWEIGHTS).
    """
    import bass_rust

    SAFE = (mybir.EngineType.Activation, mybir.EngineType.DVE, mybir.EngineType.Pool)
    insts = [i for bb in nc.main_func.blocks for i in bb.instructions]
    updaters = {}
    for i in insts:
        si = getattr(i, "sync_info", None)
        if si is None:
            continue
        for u in si.on_update:
            updaters.setdefault(u.id, set()).add(i.engine)
    for i in insts:
        if i.engine not in SAFE:
            continue
        si = getattr(i, "sync_info", None)
        if si is None or len(si.on_wait) <= 1:
            continue
        keep = [w for w in si.on_wait if updaters.get(w.id, {None}) != {i.engine}]
        if len(keep) != len(si.on_wait):
            i.sync_info = bass_rust.SyncInfo(
                on_wait=keep, on_update=list(si.on_update)
            )


def _get_nc():
    if "nc" not in _nc_cache:
        _nc_cache["nc"] = _build_nc()
    return _nc_cache["nc"]


def kernel(x, train_X, Y, W):
    global LAST_RESULTS
    x = np.ascontiguousarray(np.asarray(x, dtype=np.float32))
    train_X = np.ascontiguousarray(np.asarray(train_X, dtype=np.float32))
    Y = np.ascontiguousarray(np.asarray(Y, dtype=np.float32))
    W = np.ascontiguousarray(np.asarray(W, dtype=np.float32))

    xw = (x @ W.T).astype(np.float64)  # [B,3]
    th = (W0 * np.clip(train_X @ W.T, -P_CLIP, P_CLIP)).astype(np.float32)
    thc = (np.pi / 2 - np.abs(th)).astype(np.float32)

    in_maps = []
    for s in range(N_CORES):
        th6 = np.zeros((N_PAD, 6), dtype=np.float32)
        th6[:N_SHARD, 0:3] = thc[s * N_SHARD : (s + 1) * N_SHARD]
        th6[N_SHARD:, 0:3] = np.pi / 2
        th6[:N_SHARD, 3:6] = th[s * N_SHARD : (s + 1) * N_SHARD]
        TH = np.ascontiguousarray(th6.reshape(N_CHUNKS, CHUNK, 6).transpose(1, 0, 2))

        thr6 = np.zeros((N_PAD, 6), dtype=np.float32)
        thr6[:N_SHARD, 0:3] = thc[s * N_SHARD : (s + 1) * N_SHARD]
        thr6[:N_SHARD, 3:6] = thc[s * N_SHARD : (s + 1) * N_SHARD]
        thr6[N_SHARD:, :] = np.pi / 2
        THR = np.ascontiguousarray(
            thr6.reshape(N_CHUNKS, CHUNK, 6).transpose(1, 0, 2)
        )

        y2 = np.zeros((N_PAD, 2), dtype=np.float16)
        y2[:N_SHARD, 0] = 1.0
        y2[:N_SHARD, 1] = Y[s * N_SHARD : (s + 1) * N_SHARD].astype(np.float16)
        Y2 = np.ascontiguousarray(
            y2.reshape(N_CHUNKS, CHUNK, 2).transpose(1, 0, 2).reshape(CHUNK, -1)
        )
        in_maps.append({"TH": TH, "THR": THR, "Y2": Y2})

    nc = _get_nc()
    res = run_bass_kernel_spmd(
        nc,
        in_maps,
        core_ids=list(range(N_CORES)),
        trace=bool(int(os.environ.get("KNN_TRACE", "0"))),
    )
    LAST_RESULTS = res

    raw = np.zeros((2, ROWS), dtype=np.float64)
    for r in res.results:
        raw += r["out"].astype(np.float64)
    raw *= 0.5  # u-scaling: device rows are 2*cos / 2*sin

    mom = raw.reshape(2, M_FREQ, 2, D)  # [w, m, phi, d]

    m = np.arange(M_FREQ)
    om = m * W0
    a = (2.0 - (m == 0)) / P_PERIOD * np.sqrt(np.pi / 2.0) * np.exp(-(om**2) / 8.0)
    qarg = om[None, None, :] * xw[:, :, None]  # [B,3,M]
    qc = np.cos(qarg)
    qs = np.sin(qarg)
    S0 = np.einsum("m,bdm->bd", a, qc * mom[0, :, 0, :].T[None]) + np.einsum(
        "m,bdm->bd", a, qs * mom[0, :, 1, :].T[None]
    )
    S1 = np.einsum("m,bdm->bd", a, qc * mom[1, :, 0, :].T[None]) + np.einsum(
        "m,bdm->bd", a, qs * mom[1, :, 1, :].T[None]
    )
    return (S1 / S0).astype(np.float32)
